# revision 1
# baseline (speedup 1.0000x reference)
"""Trainium2 Bass kernel for nn_DivEncLayer (per-slice Dense->ELU->LayerNorm->Dense).

Math (per batch b, slice q):
    z[b,q,u]  = sum_s x[b, q*S+s] * W1[q,s,u]            (+ b1 via ACT bias)
    h         = elu(z + b1)
    out[b,q]  = LN(h) . (gamma*W2) + beta.W2 + b2
              = (s3 - mu*A[q]) * rsqrt(var+eps) + C[q]
where (shift-invariance of LN lets us use hh = elu+1 = max(z+b1+1, min(exp(z+b1),1)) ):
    mu  = mean_u hh,  var = mean_u hh^2 - mu^2
    s3  = sum_u G[q,u]*hh[u],  G = gamma*W2,  A[q] = sum_u G,  C[q] = sum_u beta*W2 + b2

Device layout: partitions = (q-in-group, u) / (q,s); free dim = batch.
Host pre-transposes x to xT=[Q*S, B], shards batch across 8 cores.
All LayerNorm reductions run on the TensorEngine as block-diagonal matmuls.

Perf structure (v2):
  - x / W1 / h / hsq / e and the reduction weights are all bf16, so every
    matmul streams at 1 PE cycle/col (plain f32 moving data costs 4).
  - PSUM: z pool bufs=3 (6 banks) + stats bufs=1 (2 banks); stats matmuls
    are deferred two groups behind the z matmuls so the PE never
    serializes on the ACT(exp) -> DVE(elu) elementwise chain.
  - hsq squares run on GPSIMD (TensorTensor mult), keeping ACT (exp +
    staging) and DVE (fused elu + phase 2) balanced.
  - stats leave PSUM via ACT/DVE staging copies, then scatter to the
    per-q layout over three DMA rings (SP/ACT/GPSIMD) so the last
    batch's latency overlaps; phase 2 of the final half is split into
    two column chunks to shorten the drain tail.
"""

import sys
import numpy as np

for _p in ("/opt/trn_rl_repo", "/root/.axon_site/_ro/trn_rl_repo"):
    if _p not in sys.path:
        sys.path.insert(0, _p)

Q, S, U, B = 128, 8, 32, 16384
EPS = 1e-3
NCORES = 8
BL = B // NCORES          # 2048 batch rows per core
NBH = 2                   # batch halves per core
NB = BL // NBH            # 1024 batch per chunk
NW = NB // 512            # 512-wide matmul windows per chunk
NQG = Q // 4              # 32 groups of 4 slices
NSG = Q // 16             # 8 supergroups of 16 slices (w1 packing granularity)
GPB = 10                  # q-groups batched per stats PSUM tile (12 rows each)
NBATCH = (NQG + GPB - 1) // GPB

# square-engine split, cyclic: value = engine for hsq
#   'a' = ACT (scalar.square), 'v' = DVE (tensor_mul 2x bf16), 'p' = GPSIMD
SQ_PATTERN = "pppppppppppppppp"

_compiled = None


def _register_dve_op(name, body_fn, ref_fn):
    """Create (or fetch) a custom DVE op with pinned uop shas."""
    from concourse.dve_ops import OPS, DveOp
    from concourse.dve_spec import Spec

    for op in OPS:
        if op.name == name:
            return op
    spec = Spec(body=body_fn(), reference=ref_fn)
    import concourse.dve_ops as _dops

    op = DveOp(name, spec, subdim=False, uops_sha={})
    OPS.append(op)
    _dops.CUSTOM_DVE_SPECS[op.name] = spec
    _dops._SUB_OPCODE_FOR_NAME[op.name] = _dops._CUSTOM_DVE_ROW_BASE + len(OPS) - 1
    assert _dops._SUB_OPCODE_FOR_NAME[op.name] < 0x20
    shas = {}
    for ver in ("v3", "v4"):
        try:
            op.compile(ver)
        except ValueError as e:
            import re as _re

            m = _re.search(r'uops_sha\["%s"\]="([0-9a-f]+)"' % ver, str(e))
            if m:
                shas[ver] = m.group(1)
        except Exception:
            pass
    op2 = DveOp(name, spec, subdim=False, uops_sha=shas)
    OPS[OPS.index(op)] = op2
    _dops.CUSTOM_DVE_SPECS[name] = spec
    return op2


def _elu_op():
    """out = max(in0 + s0, min(in1, 1))  (= elu(z+b1)+1 with in0=z[PSUM],
    in1=exp(z+b1)[SBUF], s0=(b1+1)[P,1])."""
    import numpy as _np
    from concourse.dve_spec import Src0, Src1, C0, One, maxx, minn

    return _register_dve_op(
        "ELU1_FUSED_ANT",
        lambda: maxx(Src0 + C0, minn(Src1, One)),
        lambda in0, in1=None, s0=0.0, s1=0.0, imm2=0.0: _np.maximum(
            in0 + s0, _np.minimum(in1, 1.0)
        ).astype(_np.float32),
    )


def _var_op():
    """out = in0 - in1^2   (var = s2 - mu^2)."""
    import numpy as _np
    from concourse.dve_spec import Src0, Src1, sq

    return _register_dve_op(
        "VAR_FUSED_ANT",
        lambda: Src0 - sq(Src1),
        lambda in0, in1=None, s0=0.0, s1=0.0, imm2=0.0: (
            in0 - in1 * in1
        ).astype(_np.float32),
    )


def _out_op():
    """out = s0 - in0*in1   (o = C - num*inv_sd, with num = mu*A - s3)."""
    import numpy as _np
    from concourse.dve_spec import Src0, Src1, C0

    return _register_dve_op(
        "OUTSTAT_FUSED_ANT",
        lambda: C0 - Src0 * Src1,
        lambda in0, in1=None, s0=0.0, s1=0.0, imm2=0.0: (
            s0 - in0 * in1
        ).astype(_np.float32),
    )


def _build(repeat=1):
    from contextlib import ExitStack
    import concourse.bacc as bacc
    import concourse.mybir as mybir
    import concourse.tile as tile

    f32 = mybir.dt.float32
    f32r = mybir.dt.float32r
    bf16 = mybir.dt.bfloat16
    Alu = mybir.AluOpType
    Act = mybir.ActivationFunctionType

    elu_op = _elu_op()
    var_op = _var_op()
    out_op = _out_op()

    nc = bacc.Bacc()
    xT = nc.declare_dram_parameter("xT", [Q * S, BL], bf16, isOutput=False)
    w1blk = nc.declare_dram_parameter("w1blk", [128, NSG * 128], bf16, isOutput=False)
    redw = nc.declare_dram_parameter("redw", [128, NQG * 128], bf16, isOutput=False)
    redwq = nc.declare_dram_parameter("redwq", [128, NQG * 128], bf16, isOutput=False)
    b1c = nc.declare_dram_parameter("b1c", [128, NQG], f32, isOutput=False)
    b1p1c = nc.declare_dram_parameter("b1p1c", [128, NQG], f32, isOutput=False)
    acol = nc.declare_dram_parameter("acol", [128, 1], f32, isOutput=False)
    ccol = nc.declare_dram_parameter("ccol", [128, 1], f32, isOutput=False)
    out = nc.declare_dram_parameter("out", [Q, BL], f32, isOutput=True)

    with ExitStack() as ctx:
        tc = ctx.enter_context(tile.TileContext(nc))
        consts = ctx.enter_context(tc.tile_pool(name="consts", bufs=1))
        xtp = ctx.enter_context(tc.tile_pool(name="xtp", bufs=2))
        sb = ctx.enter_context(tc.tile_pool(name="sb", bufs=3))
        ph = ctx.enter_context(tc.tile_pool(name="ph", bufs=2))
        psz = ctx.enter_context(tc.tile_pool(name="psz", bufs=3, space="PSUM"))
        psst = ctx.enter_context(tc.tile_pool(name="psst", bufs=1, space="PSUM"))

        w1_sb = consts.tile([128, NSG * 128], bf16)
        nc.sync.dma_start(w1_sb[:], w1blk[:])
        redw_sb = consts.tile([128, NQG * 128], bf16)
        nc.scalar.dma_start(redw_sb[:], redw[:])
        redwq_sb = consts.tile([128, NQG * 128], bf16)
        nc.scalar.dma_start(redwq_sb[:], redwq[:])
        b1_sb = consts.tile([128, NQG], f32)
        nc.sync.dma_start(b1_sb[:], b1c[:])
        b1p1_sb = consts.tile([128, NQG], f32)
        nc.sync.dma_start(b1p1_sb[:], b1p1c[:])
        a_sb = consts.tile([128, 1], f32)
        nc.sync.dma_start(a_sb[:], acol[:])
        c_sb = consts.tile([128, 1], f32)
        nc.sync.dma_start(c_sb[:], ccol[:])
        eps_sb = consts.tile([128, 1], f32)
        nc.vector.memset(eps_sb[:], EPS)

        # per-q stats accumulator: cols [mu | s3 | s2], partition = q
        s_sb = consts.tile([128, 3 * BL], f32)

        # Engine warm-ups: walrus codegen allows only ONE cross-engine
        # sync-wait on PE Matmult / ACT Activation instructions. Absorb
        # each const-DMA semaphore into each engine's vector clock with a
        # cheap op so real instructions never need two waits.
        warm_ps = psz.tile([128, 4], f32, tag="z")
        nc.tensor.matmul(
            warm_ps[:, 0:1],
            w1_sb[0:32, 0:128],
            w1_sb[0:32, 0:1],
            start=True, stop=True, tile_position=(0, 0),
        )
        wsb = consts.tile([128, 8], f32)
        nc.scalar.activation(wsb[:, 0:1], b1_sb[:, 0:1], Act.Exp)
        nc.vector.tensor_scalar_add(wsb[:, 1:2], b1p1_sb[:, 0:1], 0.0)
        nc.vector.tensor_scalar_add(wsb[:, 2:3], a_sb[:], 0.0)
        nc.vector.tensor_scalar_add(wsb[:, 3:4], c_sb[:], 0.0)

        for _rep in range(repeat):
            # whole-core input: 8 tiles of [128, BL] spanning both halves
            xts = []
            for sg in range(NSG):
                xt_sb = xtp.tile([128, BL], bf16, tag=f"xt{sg}")
                nc.sync.dma_start(xt_sb[:], xT[128 * sg : 128 * (sg + 1), :])
                xts.append(xt_sb)

            for half in range(NBH):
                pending = []  # deferred stats-matmul emitters (2-group pipe)
                st_cell = [None]

                def make_stats(g, h_sb, hsq_sb, half=half, st_cell=st_cell):
                    def emit():
                        k = g % GPB
                        ng = min(GPB, NQG - GPB * (g // GPB))
                        if k == 0:
                            st_cell[0] = psst.tile(
                                [128, NB], f32, tag="st", name="st_ps"
                            )
                        st_ps = st_cell[0]
                        for w in range(NW):
                            nc.tensor.matmul(
                                st_ps[:, 512 * w : 512 * (w + 1)],
                                redw_sb[:, 128 * g : 128 * (g + 1)],
                                h_sb[:, 512 * w : 512 * (w + 1)],
                                start=(k == 0),
                                stop=False,
                                skip_group_check=True,
                            )
                            nc.tensor.matmul(
                                st_ps[:, 512 * w : 512 * (w + 1)],
                                redwq_sb[:, 128 * g : 128 * (g + 1)],
                                hsq_sb[:, 512 * w : 512 * (w + 1)],
                                start=False,
                                stop=(k == ng - 1),
                                skip_group_check=True,
                            )
                        if k == ng - 1:
                            batch = g // GPB
                            # PSUM -> SBUF staging on ACT (cheapest per pass),
                            # then scatter stats rows (mu:0.., s3:40.., s2:80..)
                            # into per-q layout on s_sb. The last batch of a
                            # half gates phase-2: fan its scatters across three
                            # DMA rings so their latencies overlap.
                            stage = sb.tile([128, NB], f32, tag="stage", name="stage")
                            if batch % 2 == 0:
                                nc.scalar.copy(stage[:], st_ps[:])
                            else:
                                nc.vector.tensor_copy(stage[:], st_ps[:])
                            last = batch == NBATCH - 1
                            rings = (
                                (nc.sync, nc.scalar, nc.gpsimd)
                                if last
                                else (nc.sync, nc.sync, nc.sync)
                            )
                            for st in range(3):
                                src = stage[40 * st : 40 * st + 4 * ng, :]
                                dst = s_sb[
                                    40 * batch : 40 * batch + 4 * ng,
                                    st * BL + half * NB : st * BL + (half + 1) * NB,
                                ]
                                rings[st].dma_start(dst, src)

                    return emit

                for sg in range(NSG):
                    xt_sb = xts[sg]
                    for gi in range(4):
                        g = 4 * sg + gi

                        z_ps = psz.tile([128, NB], f32, tag="z")
                        for w in range(NW):
                            # fp32r: same f32 bits, 1 PE cycle/col (vs 4)
                            nc.tensor.matmul(
                                z_ps[:, 512 * w : 512 * (w + 1)],
                                w1_sb[
                                    32 * gi : 32 * (gi + 1),
                                    128 * sg : 128 * (sg + 1),
                                ],
                                xt_sb[
                                    32 * gi : 32 * (gi + 1),
                                    half * NB + 512 * w : half * NB + 512 * (w + 1),
                                ],
                                start=True,
                                stop=True,
                                tile_position=(32 * gi, 0),
                            )

                        if _rep == 0 and half == 0 and g == 0:
                            nc.tensor.matmul(
                                warm_ps[:, 1:2], redw_sb[:, 0:128],
                                redw_sb[:, 0:1], start=True, stop=True,
                            )
                            nc.tensor.matmul(
                                warm_ps[:, 2:3], redwq_sb[:, 0:128],
                                redwq_sb[:, 0:1], start=True, stop=True,
                            )
                        if len(pending) >= 2:
                            pending.pop(0)()

                        e_sb = sb.tile([128, NB], bf16, tag="e")
                        nc.scalar.activation(
                            e_sb[:], z_ps[:], Act.Exp, bias=b1_sb[:, g : g + 1]
                        )
                        h_sb = sb.tile([128, NB], bf16, tag="h")
                        # fused custom DVE op: h = max(z + b1+1, min(e,1))
                        nc.vector._custom_dve(
                            elu_op,
                            out=h_sb[:],
                            in0=z_ps[:],
                            in1=e_sb[:],
                            s0=b1p1_sb[:, g : g + 1],
                        )
                        hsq_sb = sb.tile([128, NB], bf16, tag="hsq")
                        eng = SQ_PATTERN[(32 * half + g) % len(SQ_PATTERN)]
                        if eng == "v":
                            nc.vector.tensor_mul(hsq_sb[:], h_sb[:], h_sb[:])
                        elif eng == "p":
                            nc.gpsimd.tensor_mul(hsq_sb[:], h_sb[:], h_sb[:])
                        else:
                            nc.scalar.square(hsq_sb[:], h_sb[:])

                        pending.append(make_stats(g, h_sb, hsq_sb))

                for p in pending:
                    p()

                # ---- phase 2 (per half): finalize this half's columns while
                # the other half's chunks still run (traced inline so the
                # Tile scheduler's static per-engine order interleaves them).
                nsplit = 2 if half == NBH - 1 else 1
                W2 = NB // nsplit
                for piece in range(nsplit):
                    c0 = half * NB + piece * W2
                    mu = s_sb[:, c0 : c0 + W2]
                    s3 = s_sb[:, BL + c0 : BL + c0 + W2]
                    s2 = s_sb[:, 2 * BL + c0 : 2 * BL + c0 + W2]
                    var = ph.tile([128, NB], f32, tag="var", name="var")
                    nc.vector._custom_dve(
                        var_op, out=var[:, 0:W2], in0=s2, in1=mu
                    )
                    sd = ph.tile([128, NB], f32, tag="sd", name="sd")
                    # inv_sd = rsqrt(var + eps) in one LUT pass (var+eps > 0 so
                    # abs() is a no-op; accurate enough for the 2e-2 tolerance)
                    nc.scalar.activation(
                        sd[:, 0:W2], var[:, 0:W2],
                        Act.Abs_reciprocal_sqrt, bias=eps_sb[:],
                    )
                    num = ph.tile([128, NB], f32, tag="num", name="num")
                    # num = mu*A - s3
                    nc.vector.scalar_tensor_tensor(
                        num[:, 0:W2], mu, a_sb[:], s3, Alu.mult, Alu.subtract
                    )
                    # o = C - num*inv_sd
                    o_sb = ph.tile([128, NB], f32, tag="o", name="o")
                    nc.vector._custom_dve(
                        out_op, out=o_sb[:, 0:W2], in0=num[:, 0:W2],
                        in1=sd[:, 0:W2], s0=c_sb[:],
                    )
                    nc.sync.dma_start(
                        out[:, half * NB + piece * W2 : half * NB + (piece + 1) * W2],
                        o_sb[:, 0:W2],
                    )

    nc.finalize()
    return nc


def _pack_consts(W1, b1, gamma, beta, W2, b2):
    f32 = np.float32
    W1 = np.asarray(W1, f32)
    b1 = np.asarray(b1, f32)
    gamma = np.asarray(gamma, f32)
    beta = np.asarray(beta, f32)
    W2 = np.asarray(W2, f32)
    b2 = np.asarray(b2, f32)

    G = (gamma * W2).astype(f32)               # [Q, U]
    A = G.sum(axis=1).astype(f32)              # [Q]
    C = ((beta * W2).sum(axis=1) + b2).astype(f32)

    w1blk = np.zeros((128, NSG * 128), f32)
    for sg in range(NSG):
        for gi in range(4):
            for qq in range(4):
                q = 16 * sg + 4 * gi + qq
                w1blk[
                    32 * gi + 8 * qq : 32 * gi + 8 * qq + 8,
                    128 * sg + 32 * qq : 128 * sg + 32 * qq + 32,
                ] = W1[q]

    redw = np.zeros((128, NQG * 128), f32)
    redwq = np.zeros((128, NQG * 128), f32)
    for g in range(NQG):
        k = g % GPB
        for qq in range(4):
            q = 4 * g + qq
            rows = slice(32 * qq, 32 * qq + 32)
            j = 4 * k + qq
            redw[rows, 128 * g + j] = 1.0 / U          # mu rows 0..39
            redw[rows, 128 * g + 40 + j] = G[q]        # s3 rows 40..79
            redwq[rows, 128 * g + 80 + j] = 1.0 / U    # s2 rows 80..119

    b1c = np.zeros((128, NQG), f32)
    for g in range(NQG):
        for qq in range(4):
            b1c[32 * qq : 32 * qq + 32, g] = b1[4 * g + qq]
    b1p1c = (b1c + 1.0).astype(f32)

    acol = A.reshape(128, 1)
    ccol = C.reshape(128, 1)
    import ml_dtypes
    redw = redw.astype(ml_dtypes.bfloat16)
    redwq = redwq.astype(ml_dtypes.bfloat16)
    w1blk = w1blk.astype(ml_dtypes.bfloat16)
    return dict(
        w1blk=w1blk, redw=redw, redwq=redwq, b1c=b1c, b1p1c=b1p1c,
        acol=acol, ccol=ccol,
    )


def kernel(x, W1, b1, gamma, beta, W2, b2):
    global _compiled
    from concourse import bass_utils

    import ml_dtypes
    x = np.asarray(x, np.float32).reshape(B, Q * S)
    xT = np.ascontiguousarray(x.T).astype(ml_dtypes.bfloat16)  # [Q*S, B]
    consts = _pack_consts(W1, b1, gamma, beta, W2, b2)

    if _compiled is None:
        _compiled = _build()
    nc = _compiled

    in_maps = []
    for c in range(NCORES):
        m = dict(consts)
        m["xT"] = np.ascontiguousarray(xT[:, c * BL : (c + 1) * BL])
        in_maps.append(m)

    import os
    try:
        res = bass_utils.run_bass_kernel_spmd(nc, in_maps, list(range(NCORES)))
    except Exception:
        # device may be wedged from a previous failed run; reset and retry
        os.environ["NEURON_RT_RESET_CORES"] = "1"
        res = bass_utils.run_bass_kernel_spmd(nc, in_maps, list(range(NCORES)))
    outs = [res.results[i]["out"] for i in range(NCORES)]  # each [Q, BL]
    full = np.concatenate(outs, axis=1)  # [Q, B]
    return np.ascontiguousarray(full.T).astype(np.float32)  # [B, Q]

